# revision 44
# baseline (speedup 1.0000x reference)
"""Trainium2 Bass kernel: MultiHeadLatentAttention (bf16 pipeline).

Problem (hardcoded): B=4, S=1024, HID=2048, NH=16 heads of HD=128, LAT=512,
fp32 in/out, causal attention with RoPE, latent-compressed K/V (MLA).

Sharding over 8 NeuronCores: core c = (batch b = c//2, head-group hg = c%2).
Each core handles one batch element and 8 heads (local width HL=1024).

All matmul operands are bf16 (host casts); PSUM accumulation is fp32
(bf16-everywhere measures ~5e-3 max-rel vs the 2e-2 gate).

Device layout (contraction dim always on SBUF partitions; all SBUF tiles
flat 2D [128, cols]):
  xT   [P, 16*S] bf16 (host pre-swizzled x[b].T), 4 batched DMAs
  QT = (x Wq + bq).T -> qT [P, 8*S];  latT = (x Wdown).T -> [P, 4*S]
  KT = (lat Wk_up).T -> kT [P, 8*S];  V natural -> v [P, 8*HL]
  RoPE per head-pair on [P, 2S] tiles: out = raw*cos2 + shift64(raw)*sin2e;
    the partition shift is two SBUF->SBUF DMAs issued from the SCALAR queue
    (HWDGE) so they never head-of-line-block the weight stream on Sync.
  scoresT_h = k_h @ q_h.T in [k,q] blocks; diagonal blocks column-sliced to
    widths 512/384/256/128, residual triangle zeroed by a tri mask.
  ex = exp(scores/sqrt(128)) bf16
  sums: per half-group of 4 heads one PSUM tile [4,512] accumulates
    sel-ones matmuls -> one reciprocal serves 4 heads.
  ctxT unnormalized bf16; normalized via bc = sel4^T @ rec broadcast matmul.
  out-proj of q-chunk 0 interleaved into attention of q-chunk 1.

DMA issue budget: weights/x batched into ~1MB transfers on Sync; rope
shifts + half the outT stores on Scalar (second HWDGE queue).

Host gathers: out[b] = (outT[2b] + outT[2b+1]).T + bo.
"""

import os

if "axon" not in os.environ.get("JAX_PLATFORMS", ""):
    os.environ["JAX_PLATFORMS"] = "axon"

import contextlib

import ml_dtypes
import numpy as np

import concourse.bacc as bacc
import concourse.mybir as mybir
import concourse.tile as tile
from concourse.bass_utils import run_bass_kernel_spmd

# ---- problem dims (hardcoded per contest rules)
B, S, HID, NH, LAT = 4, 1024, 2048, 16, 512
HD = 128
NHL = NH // 2          # heads per core = 8
HL = NHL * HD          # local head width = 1024
P = 128
KT_H = HID // P        # 16
KT_L = LAT // P        # 4
QCW = 512              # q-chunk width (PSUM bank = 512 fp32)
NQC = S // QCW         # 2
SC_SCALE = float(1.0 / np.sqrt(HD))

F32 = mybir.dt.float32
BF16 = mybir.dt.bfloat16
NPBF = ml_dtypes.bfloat16

N_CORES = 8
CPACK_W = 2 * S + 2 * S + P + 4 * P   # cos2 | sin2 | tri | selones128


def build_bass(loop_iters=None):
    nc = bacc.Bacc("TRN2", target_bir_lowering=False, debug=False, num_devices=8)

    xTd = nc.dram_tensor("xT", [P, KT_H, S], BF16, kind="ExternalInput")[:]
    wqd = nc.dram_tensor("wq", [P, KT_H, HL], BF16, kind="ExternalInput")[:]
    wdownd = nc.dram_tensor("wdown", [P, KT_H, LAT], BF16, kind="ExternalInput")[:]
    wkupd = nc.dram_tensor("wkup", [P, KT_L, HL], BF16, kind="ExternalInput")[:]
    wvupd = nc.dram_tensor("wvup", [P, KT_L, HL], BF16, kind="ExternalInput")[:]
    wod = nc.dram_tensor("wo", [P, NHL, HID], BF16, kind="ExternalInput")[:]
    bqd = nc.dram_tensor("bq", [P, NHL], F32, kind="ExternalInput")[:]
    cpackd = nc.dram_tensor("cpack", [P, CPACK_W], BF16, kind="ExternalInput")[:]
    sel4d = nc.dram_tensor("sel4", [4, 4 * P], BF16, kind="ExternalInput")[:]
    outTd = nc.dram_tensor("outT", [HID, S], F32, kind="ExternalOutput")[:]

    with tile.TileContext(nc) as tc, contextlib.ExitStack() as _les:
        if loop_iters is not None:
            _les.enter_context(tc.For_i(0, loop_iters, 1))
        with (
            tc.tile_pool(name="consts", bufs=1) as consts,
            tc.tile_pool(name="resident", bufs=1) as resident,
        ):
            cpack = consts.tile([P, CPACK_W], BF16)
            cos2_sb = cpack[:, 0:2 * S]
            sin2_sb = cpack[:, 2 * S:4 * S]
            tri_sb = cpack[:, 4 * S:4 * S + P]
            selo_sb = cpack[:, 4 * S + P:4 * S + P + 4 * P]
            bq_sb = consts.tile([P, NHL], F32)
            sel4_sb = consts.tile([4, 4 * P], BF16)

            latT = resident.tile([P, KT_L * S], BF16)
            qT = resident.tile([P, NHL * S], BF16)
            kT = resident.tile([P, NHL * S], BF16)
            v_sb = resident.tile([P, NHL * HL], BF16)
            ctxT = resident.tile([P, NHL * S], BF16)
            # phase-B weights, loaded during phase A (wvg also feeds the
            # V hl-half-1 filler inside phase C)
            wkg = resident.tile([P, KT_L * HL], BF16)
            wvg = resident.tile([P, KT_L * HL], BF16)

            pacc_cm = tc.tile_pool(name="pacc", bufs=8, space="PSUM")
            pacc = pacc_cm.__enter__()

            def rope_pair(rp, h, ps4, dst, bias, sin_eng,
                          add_eng=None, dma_eng=None):
                """RoPE for heads h, h+1 from 4 psum tiles [(j,ntc)]."""
                add_eng = add_eng or nc.vector
                dma_eng = dma_eng or nc.scalar
                raw = rp.tile([P, 2 * S], BF16, tag="raw", name="raw")
                sh = rp.tile([P, 2 * S], BF16, tag="sh", name="sh")
                for j in range(2):
                    for ntc in range(NQC):
                        seg = raw[:, (j * NQC + ntc) * QCW:
                                  (j * NQC + ntc + 1) * QCW]
                        if bias:
                            nc.scalar.add(seg, ps4[j * 2 + ntc],
                                          bq_sb[:, h + j:h + j + 1])
                        else:
                            nc.scalar.copy(seg, ps4[j * 2 + ntc])
                    # per-head shift: unblocks as soon as this head's two
                    # segment copies land (not the whole pair)
                    dma_eng.dma_start(sh[0:64, j * S:(j + 1) * S],
                                      raw[64:128, j * S:(j + 1) * S])
                    dma_eng.dma_start(sh[64:128, j * S:(j + 1) * S],
                                      raw[0:64, j * S:(j + 1) * S])
                out = dst[:, h * S:(h + 2) * S]
                nc.vector.tensor_mul(out, raw, cos2_sb)
                sin_eng.tensor_mul(sh, sh, sin2_sb)
                add_eng.tensor_add(out, out, sh)

            # ---------- phase A: QT (2 groups of 4 heads) + latT ----------
            with (
                tc.tile_pool(name="xp", bufs=1) as xp,
                tc.tile_pool(name="ws1", bufs=2) as ws1,
                tc.tile_pool(name="ropeA", bufs=2) as rpA,
            ):
                xT_sb = xp.tile([P, KT_H * S], BF16)
                # ramp-in: first x chunk + first weight chunk land ASAP so
                # the PE starts ~4us in, then the bulk streams behind them
                wg0 = ws1.tile([P, KT_H * QCW], BF16, tag="w", name="wg")
                nc.sync.dma_start(xT_sb[:, 0:2 * S], xTd[:, 0:2, :])
                nc.sync.dma_start(wg0[:, 0:4 * QCW], wqd[:, 0:4, 0:QCW])
                nc.sync.dma_start(xT_sb[:, 2 * S:4 * S], xTd[:, 2:4, :])
                nc.sync.dma_start(wg0[:, 4 * QCW:8 * QCW],
                                  wqd[:, 4:8, 0:QCW])
                nc.sync.dma_start(xT_sb[:, 4 * S:8 * S], xTd[:, 4:8, :])
                nc.sync.dma_start(wg0[:, 8 * QCW:16 * QCW],
                                  wqd[:, 8:16, 0:QCW])
                nc.sync.dma_start(xT_sb[:, 8 * S:16 * S], xTd[:, 8:16, :])
                nc.sync.dma_start(bq_sb, bqd)
                # preload the exp table set while the PE is busy with
                # projections (first ACTIVATE otherwise pays ~2.7us in C)
                warm = ws1.tile([1, NHL], F32, tag="warm", name="warm")
                nc.scalar.activation(
                    out=warm, in_=bq_sb[0:1, :],
                    func=mybir.ActivationFunctionType.Exp, scale=1.0)

                for og in range(2):
                    if og == 0:
                        wg = wg0
                    else:
                        wg = ws1.tile([P, KT_H * QCW], BF16, tag="w",
                                      name="wg")
                        for hf in range(2):   # two 1MB halves
                            nc.sync.dma_start(
                                wg[:, hf * 8 * QCW:(hf + 1) * 8 * QCW],
                                wqd[:, hf * 8:(hf + 1) * 8,
                                    og * QCW:(og + 1) * QCW])
                    ps = [pacc.tile([P, QCW], F32, tag="acc", name="acc")
                          for _ in range(8)]
                    for kt in range(KT_H):
                        for oi in range(4):
                            for ntc in range(NQC):
                                nc.tensor.matmul(
                                    ps[oi * 2 + ntc],
                                    lhsT=wg[:, kt * QCW + oi * P:
                                            kt * QCW + (oi + 1) * P],
                                    rhs=xT_sb[:, kt * S + ntc * QCW:
                                              kt * S + (ntc + 1) * QCW],
                                    start=(kt == 0),
                                    stop=(kt == KT_H - 1),
                                )
                    if og == 0:
                        nc.sync.dma_start(cpack, cpackd)
                        nc.sync.dma_start(sel4_sb, sel4d)
                    for pr in range(2):
                        rope_pair(rpA, og * 4 + pr * 2,
                                  ps[pr * 4:pr * 4 + 4], qT, bias=True,
                                  sin_eng=nc.gpsimd)

                # latT group (4 out tiles x 2 chunks)
                wg = ws1.tile([P, KT_H * QCW], BF16, tag="w", name="wg")
                for hf in range(2):
                    nc.sync.dma_start(
                        wg[:, hf * 8 * QCW:(hf + 1) * 8 * QCW],
                        wdownd[:, hf * 8:(hf + 1) * 8, :])
                nc.sync.dma_start(wkg, wkupd)
                nc.sync.dma_start(wvg, wvupd)
                ps = [pacc.tile([P, QCW], F32, tag="acc", name="acc")
                      for _ in range(8)]
                for kt in range(KT_H):
                    for oi in range(4):
                        for ntc in range(NQC):
                            nc.tensor.matmul(
                                ps[oi * 2 + ntc],
                                lhsT=wg[:, kt * QCW + oi * P:
                                        kt * QCW + (oi + 1) * P],
                                rhs=xT_sb[:, kt * S + ntc * QCW:
                                          kt * S + (ntc + 1) * QCW],
                                start=(kt == 0),
                                stop=(kt == KT_H - 1),
                            )
                for oi in range(4):
                    for ntc in range(NQC):
                        dstap = latT[:, oi * S + ntc * QCW:
                                     oi * S + (ntc + 1) * QCW]
                        if ntc == 0:
                            nc.scalar.copy(dstap, ps[oi * 2 + ntc])
                        else:
                            nc.vector.tensor_copy(dstap, ps[oi * 2 + ntc])

            # ---------- phase B: KT (rope) then V hl-half 0 ----------
            # (wkup/wvup were loaded during phase A; wo loads during B;
            #  V hl-half 1 is emitted later as PE filler inside qc0
            #  attention, using the pctx pool.)
            wop_cm = tc.tile_pool(name="wop", bufs=1)
            wop = wop_cm.__enter__()
            wo_sb = wop.tile([P, NHL * HID], BF16)
            nc.sync.dma_start(wo_sb, wod)

            rpB_cm = tc.tile_pool(name="ropeB", bufs=2)
            rpB = rpB_cm.__enter__()

            for og in range(2):
                ps = [pacc.tile([P, QCW], F32, tag="acc", name="acc")
                      for _ in range(8)]
                for kt in range(KT_L):
                    for oi in range(4):
                        for ntc in range(NQC):
                            nc.tensor.matmul(
                                ps[oi * 2 + ntc],
                                lhsT=wkg[:, kt * HL + og * 4 * P + oi * P:
                                         kt * HL + og * 4 * P
                                         + (oi + 1) * P],
                                rhs=latT[:, kt * S + ntc * QCW:
                                         kt * S + (ntc + 1) * QCW],
                                start=(kt == 0),
                                stop=(kt == KT_L - 1),
                            )
                for pr in range(2):
                    sin = nc.vector if og == 0 else nc.gpsimd
                    rope_pair(rpB, og * 4 + pr * 2,
                              ps[pr * 4:pr * 4 + 4], kT, bias=False,
                              sin_eng=sin, add_eng=nc.vector,
                              dma_eng=nc.sync)

            for hlc in range(2):
                ps = [pacc.tile([P, QCW], F32, tag="acc", name="acc")
                      for _ in range(8)]
                for kt in range(KT_L):
                    for st in range(8):
                        nc.tensor.matmul(
                            ps[st],
                            lhsT=latT[:, kt * S + st * P:
                                      kt * S + (st + 1) * P],
                            rhs=wvg[:, kt * HL + hlc * QCW:
                                    kt * HL + (hlc + 1) * QCW],
                            start=(kt == 0),
                            stop=(kt == KT_L - 1),
                        )
                for st in range(8):
                    dstap = v_sb[:, st * HL + hlc * QCW:
                                 st * HL + (hlc + 1) * QCW]
                    if st % 4 == 1:
                        nc.vector.tensor_copy(dstap, ps[st])
                    else:
                        nc.scalar.copy(dstap, ps[st])

            rpB_cm.__exit__(None, None, None)
            pacc_cm.__exit__(None, None, None)

            # ---------- phase C: attention + out-projection ----------
            with (
                tc.tile_pool(name="psc", bufs=2, space="PSUM") as psc,
                tc.tile_pool(name="pctx", bufs=2, space="PSUM") as pctx,
                tc.tile_pool(name="psums", bufs=2, space="PSUM") as psums,
                tc.tile_pool(name="pbc", bufs=2, space="PSUM") as pbc,
                tc.tile_pool(name="exla", bufs=3) as exla,
                tc.tile_pool(name="exlb", bufs=3) as exlb,
                tc.tile_pool(name="small", bufs=2) as small,
                tc.tile_pool(name="outsb", bufs=3) as outsb,
            ):
                # ---- fine-grained PE filler: each closure emits ~one
                # always-ready matmul (out-proj accumulation step or a
                # V hl-half-1 accumulation step), popped between attention
                # dependency steps to keep the PE dense and warm.
                fill = []

                def fl_pop(n):
                    for _ in range(n):
                        if fill:
                            fill.pop(0)()

                def add_outproj(qc, ot):
                    st_ = {}

                    def mk_mm(kt):
                        def go():
                            if kt == 0:
                                st_["po"] = pbc.tile([P, QCW], F32,
                                                     tag="bcpo", name="po")
                            nc.tensor.matmul(
                                st_["po"],
                                lhsT=wo_sb[:, kt * HID + ot * P:
                                           kt * HID + (ot + 1) * P],
                                rhs=ctxT[:, kt * S + qc * QCW:
                                         kt * S + (qc + 1) * QCW],
                                start=(kt == 0),
                                stop=(kt == NHL - 1),
                            )
                        return go

                    def fin():
                        ob = outsb.tile([P, QCW], F32, tag="osb", name="ob")
                        nc.vector.tensor_copy(ob, st_["po"])
                        # last stores of the iteration leave on Scalar so
                        # Sync reaches the next iteration's x/weight stream
                        # sooner (Scalar's head work isn't needed for ~30us)
                        eng = nc.scalar if (qc == 1 and ot >= 10) else nc.sync
                        eng.dma_start(
                            outTd[ot * P:(ot + 1) * P,
                                  qc * QCW:(qc + 1) * QCW], ob)

                    for kt in range(NHL):
                        fill.append(mk_mm(kt))
                    fill.append(fin)


                def att_unit(h, hh, qc, sums_ps):
                    """Generator: one attention head, yields per kt step."""
                    nkt = 4 * qc + 4
                    ctx = pctx.tile([P, QCW], F32, tag="ctx", name="ctx")

                    def geom(kt):
                        off = kt - 4 * qc
                        if off < 0:
                            return 0, QCW, False
                        return 128 * off, QCW - 128 * off, True

                    def emit_sc(kt):
                        lo, w, diag = geom(kt)
                        sc = psc.tile([P, QCW], F32, tag="sc", name="sc")
                        nc.tensor.matmul(
                            sc[:, :w],
                            lhsT=kT[:, h * S + kt * P:h * S + (kt + 1) * P],
                            rhs=qT[:, h * S + qc * QCW + lo:
                                   h * S + qc * QCW + lo + w],
                            start=True, stop=True,
                        )
                        exp_pool = exla if kt % 2 == 0 else exlb
                        ex = exp_pool.tile([P, QCW], BF16, tag="ex",
                                           name="ex")
                        nc.scalar.activation(
                            out=ex[:, :w], in_=sc[:, :w],
                            func=mybir.ActivationFunctionType.Exp,
                            scale=SC_SCALE,
                        )
                        if diag:
                            # gpsimd only late in C (it chews kT h4-7 rope
                            # early on)
                            eng = (nc.gpsimd if (qc == 1 and h >= 4
                                                 and kt % 2 == 0)
                                   else nc.vector)
                            eng.tensor_mul(ex[:, 0:P], ex[:, 0:P], tri_sb)
                        return ex

                    def emit_pv(kt, ex):
                        lo, w, _ = geom(kt)
                        nc.tensor.matmul(
                            ctx[:, lo:lo + w],
                            lhsT=v_sb[:, kt * HL + h * P:
                                      kt * HL + (h + 1) * P],
                            rhs=ex[:, :w],
                            start=(kt == 0),
                            stop=(kt == nkt - 1),
                        )
                        nc.tensor.matmul(
                            sums_ps[:, lo:lo + w],
                            lhsT=selo_sb[:, hh * P:(hh + 1) * P],
                            rhs=ex[:, :w],
                            start=(hh == 0 and kt == 0),
                            stop=(hh == 3 and kt == nkt - 1),
                        )

                    exs = {0: emit_sc(0)}
                    for kt in range(nkt):
                        if kt + 1 < nkt:
                            exs[kt + 1] = emit_sc(kt + 1)
                        emit_pv(kt, exs.pop(kt))
                        yield
                    ctx_dst = ctxT[:, h * S + qc * QCW:
                                   h * S + (qc + 1) * QCW]
                    if qc == 0:
                        nc.scalar.copy(ctx_dst, ctx)
                    else:
                        nc.vector.tensor_copy(ctx_dst, ctx)

                def drive(gens, on_done=None):
                    live = list(gens)
                    while live:
                        nxt = []
                        for g in live:
                            try:
                                next(g)
                                nxt.append(g)
                            except StopIteration:
                                if on_done is not None:
                                    on_done(g)
                            fl_pop(1)
                        live = nxt

                pending_norm = []

                def flush_norm():
                    while pending_norm:
                        pending_norm.pop(0)()

                def finish_group(qc, half, sums_ps):
                    # copy the sums rows out to free the PSUM bank fast,
                    # then reciprocal off the SBUF copy
                    srow = small.tile([4, QCW], F32, tag="srow",
                                      name="srow")
                    nc.vector.tensor_copy(srow, sums_ps[0:4, :])
                    rec = small.tile([4, QCW], BF16, tag="rec", name="rec")
                    with nc.allow_low_precision(reason="bf16 softmax "
                                                "denominator (gate 2e-2)"):
                        nc.vector.reciprocal(out=rec, in_=srow)

                    def go():
                        for hh in range(4):
                            # filler first: the bc matmul may wait on the
                            # reciprocal, so give the PE ready work ahead
                            fl_pop(3)
                            h = half * 4 + hh
                            bc = pbc.tile([P, QCW], F32, tag="bcpo",
                                          name="bc")
                            nc.tensor.matmul(
                                bc,
                                lhsT=sel4_sb[:, hh * P:(hh + 1) * P],
                                rhs=rec,
                                start=True, stop=True,
                            )
                            sl = ctxT[:, h * S + qc * QCW:
                                      h * S + (qc + 1) * QCW]
                            nc.vector.tensor_mul(sl, sl, bc)
                    return go

                # two passes: (qc0 h || qc1 h) paired per head — the short
                # qc0 stream and long qc1 stream hide each other's
                # dependency stalls on the in-order PE queue
                for half in range(2):
                    sums_a = psums.tile([P, QCW], F32, tag="sums",
                                        name="sums")
                    sums_b = psums.tile([P, QCW], F32, tag="sums",
                                        name="sums")
                    for hh in range(4):
                        h = half * 4 + hh
                        drive([att_unit(h, hh, 0, sums_a),
                               att_unit(h, hh, 1, sums_b)])
                        if hh == 0:
                            flush_norm()
                    norm_a = finish_group(0, half, sums_a)
                    norm_b = finish_group(1, half, sums_b)
                    if half == 0:
                        pending_norm += [norm_a, norm_b]
                    else:
                        norm_a()
                        for ot in range(HID // P):
                            add_outproj(0, ot)
                        fl_pop(12)   # cover norm_b's reciprocal wait
                        norm_b()
                        for ot in range(HID // P):
                            add_outproj(1, ot)
                while fill:
                    fill.pop(0)()

            wop_cm.__exit__(None, None, None)
    nc.compile()
    return nc


# ---------------- host side ----------------

def _host_consts():
    inv_freq = 1.0 / (10000.0 ** (np.arange(0, HD, 2, dtype=np.float64) / HD))
    t = np.arange(S, dtype=np.float64)
    freqs = t[:, None] * inv_freq[None, :]            # [S, 64]
    emb = np.concatenate([freqs, freqs], axis=-1)     # [S, 128]
    cosT = np.cos(emb).T.astype(np.float32)           # [128, S]
    sinT = np.sin(emb).T.astype(np.float32)
    sinTe = sinT.copy()
    sinTe[:64] *= -1.0                                # rotate_half sign folded
    cos2 = np.broadcast_to(cosT[:, None, :], (P, 2, S)).reshape(P, 2 * S)
    sin2 = np.broadcast_to(sinTe[:, None, :], (P, 2, S)).reshape(P, 2 * S)

    ii = np.arange(P)[:, None]
    tri = (np.arange(P)[None, :] - ii >= 0).astype(np.float32)  # [128,128]

    selones = np.zeros((P, 4 * P), dtype=np.float32)
    for hh in range(4):
        selones[:, hh * P + hh] = 1.0
    cpack = np.ascontiguousarray(
        np.concatenate([cos2, sin2, tri, selones], axis=1)).astype(NPBF)

    sel4 = np.zeros((4, 4 * P), dtype=NPBF)
    for hh in range(4):
        sel4[hh, hh * P:(hh + 1) * P] = 1.0
    return cpack, sel4


_CACHE = {}


def _get_built():
    if "nc" not in _CACHE:
        _CACHE["nc"] = build_bass()
        _CACHE["consts"] = _host_consts()
    return _CACHE["nc"], _CACHE["consts"]


def _swz(a, n_kt):
    """[n_kt*128, W] -> [128, n_kt, W] (partition-major swizzle), bf16."""
    w = a.shape[1]
    return np.ascontiguousarray(
        a.reshape(n_kt, P, w).transpose(1, 0, 2)).astype(NPBF)


def make_in_maps(x, Wq, bq, Wdown, Wk_up, Wv_up, Wo):
    cpack, sel4 = _get_built()[1]
    in_maps = []
    for c in range(N_CORES):
        b, hg = c // 2, c % 2
        sl = slice(hg * HL, (hg + 1) * HL)
        in_maps.append({
            "xT": _swz(np.ascontiguousarray(x[b].T), KT_H),
            "wq": _swz(Wq[:, sl], KT_H),
            "wdown": _swz(Wdown, KT_H),
            "wkup": _swz(Wk_up[:, sl], KT_L),
            "wvup": _swz(Wv_up[:, sl], KT_L),
            "wo": _swz(Wo[sl, :], NHL),
            "bq": np.ascontiguousarray(
                bq[sl].reshape(NHL, P).T).astype(np.float32),
            "cpack": cpack,
            "sel4": sel4,
        })
    return in_maps


def gather_out(results, bo):
    out = np.empty((B, S, HID), dtype=np.float32)
    for b in range(B):
        acc = results[2 * b]["outT"] + results[2 * b + 1]["outT"]  # [HID, S]
        out[b] = acc.T + bo[None, :]
    return out


def kernel(x, Wq, bq, Wdown, Wk_up, Wv_up, Wo, bo):
    x = np.asarray(x, dtype=np.float32)
    Wq = np.asarray(Wq, dtype=np.float32)
    bq = np.asarray(bq, dtype=np.float32)
    Wdown = np.asarray(Wdown, dtype=np.float32)
    Wk_up = np.asarray(Wk_up, dtype=np.float32)
    Wv_up = np.asarray(Wv_up, dtype=np.float32)
    Wo = np.asarray(Wo, dtype=np.float32)
    bo = np.asarray(bo, dtype=np.float32)

    nc, _ = _get_built()
    in_maps = make_in_maps(x, Wq, bq, Wdown, Wk_up, Wv_up, Wo)
    res = run_bass_kernel_spmd(nc, in_maps, core_ids=list(range(N_CORES)))
    return gather_out(res.results, bo)
